# revision 5
# baseline (speedup 1.0000x reference)
"""Trainium2 Bass kernel for nn_AdvResNet (dense_mlp, 8 NeuronCores).

Reference math (adv=1 path, the one setup_inputs produces):
    beta_norm[n] = sum_k |beta[k, n]|                       # [1024]
    one[n]      = 4096 * sum_h W2[n, h] + bias2[n]          # [1024]
    out[b, n]   = (x @ beta)[b, n] + bias_lin[n]
                  - 0.1 * y[b, n] * beta_norm[n] + one[n]

The x@W1 relu MLP is dead code when adv=1, so W1/bias1 never touch the
device.

Distribution: data-parallel over batch (512 rows/core), beta replicated.
Each core computes in TRANSPOSED layout: outT = beta^T @ x^T via
matmul(psum[n,b], lhsT=beta[k,n] (natural layout), rhs=xT[k,b]), so the
per-n vectors (beta_norm, one, biases) are per-partition scalars, which
feed the scalar-engine activation(scale*in+bias) directly.

The beta_norm / W2-rowsum reductions are sharded 8-ways (each core
reduces a 1/8 slice along the contraction axis with a free-axis
vector-reduce, abs fused) and combined with a single 8KB AllReduce.

Matmuls run in float32r (fp32 operands, 1 cycle/row at N=512).
"""

import os
import sys

sys.path.insert(0, "/opt/trn_rl_repo")
os.environ.setdefault("NEURON_RT_RESET_CORES", "1")

import numpy as np

import concourse.bass as bass  # noqa: F401
import concourse.tile as tile
from concourse import bacc, mybir
from concourse.bass_utils import run_bass_kernel_spmd

B, NIN, NHID, NOUT = 4096, 2048, 4096, 1024
NC = 8
BS = B // NC  # 512 batch rows per core
KT = NIN // 128  # 16 k-tiles
NT = NOUT // 128  # 8 n-tiles
KSH = NIN // NC  # 256: beta_norm k-slice per core
HSH = NHID // NC  # 512: W2 h-slice per core
EPS = 0.1
F32 = mybir.dt.float32
F32R = mybir.dt.float32r

_CACHE = {}


def build_bass():
    nc = bacc.Bacc("TRN2", target_bir_lowering=False, debug=False, num_devices=NC)

    xT = nc.declare_dram_parameter("xT", [NIN, BS], F32, isOutput=False)
    yT = nc.declare_dram_parameter("yT", [NOUT, BS], F32, isOutput=False)
    bet = nc.declare_dram_parameter("beta", [NIN, NOUT], F32, isOutput=False)
    btp = nc.declare_dram_parameter("btp", [128, NT, KSH], F32, isOutput=False)
    w2p = nc.declare_dram_parameter("w2p", [128, NT, HSH], F32, isOutput=False)
    blp = nc.declare_dram_parameter("blp", [128, NT], F32, isOutput=False)
    b2p = nc.declare_dram_parameter("b2p", [128, NT], F32, isOutput=False)
    out = nc.declare_dram_parameter("out", [NOUT, BS], F32, isOutput=True)

    with (
        tile.TileContext(nc) as tc,
        tc.tile_pool(name="bsb", bufs=KT) as bpool,
        tc.tile_pool(name="xsb", bufs=KT) as xpool,
        tc.tile_pool(name="yts", bufs=NT) as ypool,
        tc.tile_pool(name="aux", bufs=1) as aux,
        tc.tile_pool(name="psum", bufs=1, space="PSUM") as ppool,
        tc.tile_pool(name="dram", bufs=1, space="DRAM") as dpool,
    ):
        ps = [
            ppool.tile([128, BS], F32, name=f"ps{n}", tag=f"ps{n}")
            for n in range(NT)
        ]

        def k_step(k):
            bt = bpool.tile([128, NOUT], F32R, tag="bt")
            nc.sync.dma_start(
                out=bt[:], in_=bet[k * 128 : (k + 1) * 128, :].bitcast(F32R)
            )
            xt = xpool.tile([128, BS], F32R, tag="xt")
            nc.sync.dma_start(
                out=xt[:], in_=xT[k * 128 : (k + 1) * 128, :].bitcast(F32R)
            )
            for n in range(NT):
                nc.tensor.matmul(
                    ps[n][:],
                    lhsT=bt[:, n * 128 : (n + 1) * 128],
                    rhs=xt[:],
                    start=(k == 0),
                    stop=(k == KT - 1),
                )

        # First two k-steps start the matmul stream immediately.
        k_step(0)
        k_step(1)

        # Sharded beta_norm / W2-rowsum partials + AllReduce (emitted here
        # so their DMAs issue early and the collective hides under the
        # matmul stream).
        w2s = aux.tile([128, NT, HSH], F32)
        nc.sync.dma_start(out=w2s[:], in_=w2p[:])
        bts = aux.tile([128, NT, KSH], F32)
        nc.sync.dma_start(out=bts[:], in_=btp[:])
        part = aux.tile([128, 2 * NT], F32)
        nc.vector.tensor_reduce(
            out=part[:, 0:NT],
            in_=w2s[:],
            axis=mybir.AxisListType.X,
            op=mybir.AluOpType.add,
        )
        nc.vector.tensor_reduce(
            out=part[:, NT : 2 * NT],
            in_=bts[:],
            axis=mybir.AxisListType.X,
            op=mybir.AluOpType.add,
            apply_absolute_value=True,
        )
        cin = dpool.tile([128, 2 * NT], F32)
        cout = dpool.tile([128, 2 * NT], F32)
        nc.sync.dma_start(out=cin[:], in_=part[:])
        nc.gpsimd.collective_compute(
            "AllReduce",
            mybir.AluOpType.add,
            replica_groups=[list(range(NC))],
            ins=[cin.opt()],
            outs=[cout.opt()],
        )
        allred = aux.tile([128, 2 * NT], F32)
        nc.sync.dma_start(out=allred[:], in_=cout[:])

        # scale[n] = -EPS * beta_norm[n];  biasc[n] = NHID*w2sum + bias2 + bias_lin
        blt = aux.tile([128, NT], F32)
        nc.sync.dma_start(out=blt[:], in_=blp[:])
        b2t = aux.tile([128, NT], F32)
        nc.sync.dma_start(out=b2t[:], in_=b2p[:])
        scale = aux.tile([128, NT], F32)
        nc.vector.tensor_scalar_mul(scale[:], allred[:, NT : 2 * NT], -EPS)
        biasc = aux.tile([128, NT], F32)
        nc.vector.tensor_scalar_mul(biasc[:], allred[:, 0:NT], float(NHID))
        nc.vector.tensor_add(biasc[:], biasc[:], b2t[:])
        nc.vector.tensor_add(biasc[:], biasc[:], blt[:])

        # Rest of the matmul stream.
        for k in range(2, KT):
            k_step(k)

        # t[n] = yT*scale + biasc precomputed on the scalar engine (no
        # dependence on the matmuls, fills ACT idle time).
        yts = []
        for n in range(NT):
            yt = ypool.tile([128, BS], F32, tag="yt")
            nc.sync.dma_start(out=yt[:], in_=yT[n * 128 : (n + 1) * 128, :])
            nc.scalar.activation(
                yt[:],
                yt[:],
                mybir.ActivationFunctionType.Identity,
                bias=biasc[:, n : n + 1],
                scale=scale[:, n : n + 1],
            )
            yts.append(yt)

        # Epilogue: out = psum(lin^T) + t, then store.
        for n in range(NT):
            nc.vector.tensor_add(yts[n][:], ps[n][:], yts[n][:])
            nc.sync.dma_start(out=out[n * 128 : (n + 1) * 128, :], in_=yts[n][:])

    nc.compile()
    return nc


def _get_nc():
    if "nc" not in _CACHE:
        _CACHE["nc"] = build_bass()
    return _CACHE["nc"]


def _shard_inputs(x, y, beta, bias_lin, W2, bias2):
    x = np.ascontiguousarray(x, dtype=np.float32)
    y = np.ascontiguousarray(y, dtype=np.float32)
    beta = np.ascontiguousarray(beta, dtype=np.float32)
    W2 = np.ascontiguousarray(W2, dtype=np.float32)
    blp = np.ascontiguousarray(np.asarray(bias_lin, np.float32).reshape(NT, 128).T)
    b2p = np.ascontiguousarray(np.asarray(bias2, np.float32).reshape(NT, 128).T)
    betaT = np.ascontiguousarray(beta.T)  # [NOUT, NIN]
    in_maps = []
    for c in range(NC):
        bsl = slice(c * BS, (c + 1) * BS)
        # [128, NT, KSH]: btp[p, t, k] = |slice later| beta[c*KSH+k, t*128+p]
        btp = np.ascontiguousarray(
            betaT[:, c * KSH : (c + 1) * KSH]
            .reshape(NT, 128, KSH)
            .transpose(1, 0, 2)
        )
        w2p = np.ascontiguousarray(
            W2[:, c * HSH : (c + 1) * HSH].reshape(NT, 128, HSH).transpose(1, 0, 2)
        )
        in_maps.append(
            {
                "xT": np.ascontiguousarray(x[bsl].T),
                "yT": np.ascontiguousarray(y[bsl].T),
                "beta": beta,
                "btp": btp,
                "w2p": w2p,
                "blp": blp,
                "b2p": b2p,
            }
        )
    return in_maps


def run_device(inputs, trace=False, **kw):
    nc = _get_nc()
    in_maps = _shard_inputs(
        inputs["x"], inputs["y"], inputs["beta"], inputs["bias_lin"],
        inputs["W2"], inputs["bias2"],
    )
    res = run_bass_kernel_spmd(nc, in_maps, core_ids=list(range(NC)), trace=trace, **kw)
    full = np.empty((B, NOUT), dtype=np.float32)
    for c in range(NC):
        full[c * BS : (c + 1) * BS, :] = res.results[c]["out"].T
    return full, res


def _reference_numpy(x, y, beta, bias_lin, W1, W2, bias1, bias2, adv):
    # Fallback for the adv=0 path (never produced by setup_inputs).
    x = np.asarray(x, np.float32)
    lin = x @ np.asarray(beta, np.float32) + np.asarray(bias_lin, np.float32)
    if adv:
        beta_norm = np.sum(np.abs(np.asarray(beta, np.float32)), axis=0)
        lin = lin - EPS * np.asarray(y, np.float32) * beta_norm
        one = NHID * np.sum(np.asarray(W2, np.float32), axis=1) + np.asarray(
            bias2, np.float32
        )
        one = np.broadcast_to(one, lin.shape)
    else:
        h = np.maximum(
            x @ np.asarray(W1, np.float32).T + np.asarray(bias1, np.float32), 0.0
        )
        one = h @ np.asarray(W2, np.float32).T + np.asarray(bias2, np.float32)
    return (lin + one).astype(np.float32)


def kernel(**inputs) -> np.ndarray:
    adv = int(np.asarray(inputs.get("adv", 1)))
    if adv == 0:
        return _reference_numpy(
            inputs["x"], inputs["y"], inputs["beta"], inputs["bias_lin"],
            inputs["W1"], inputs["W2"], inputs["bias1"], inputs["bias2"], adv,
        )
    full, _ = run_device(inputs)
    return full


# revision 6
# speedup vs baseline: 1.1178x; 1.1178x over previous
"""Trainium2 Bass kernel for nn_AdvResNet (dense_mlp, 8 NeuronCores).

Reference math (adv=1 path, the one setup_inputs produces):
    beta_norm[n] = sum_k |beta[k, n]|                       # [1024]
    one[n]      = 4096 * sum_h W2[n, h] + bias2[n]          # [1024]
    out[b, n]   = (x @ beta)[b, n] + bias_lin[n]
                  - 0.1 * y[b, n] * beta_norm[n] + one[n]

The x@W1 relu MLP is dead code when adv=1, so W1/bias1 never touch the
device.

Distribution: data-parallel over batch (512 rows/core), beta replicated.
Each core computes in TRANSPOSED layout: outT = beta^T @ x^T via
matmul(psum[n,b], lhsT=beta[k,n] (natural layout), rhs=xT[k,b]), so the
per-n vectors (beta_norm, one, biases) are per-partition scalars, which
feed the scalar-engine activation(scale*in+bias) directly.

The beta_norm / W2-rowsum reductions are sharded 8-ways (each core
reduces a 1/8 slice along the contraction axis with a free-axis
vector-reduce, abs fused) and combined with a single 8KB AllReduce.

Matmuls run in float32r (fp32 operands, 1 cycle/row at N=512).
"""

import os
import sys

sys.path.insert(0, "/opt/trn_rl_repo")
os.environ.setdefault("NEURON_RT_RESET_CORES", "1")

import numpy as np

import concourse.bass as bass  # noqa: F401
import concourse.tile as tile
from concourse import bacc, mybir
from concourse.bass_utils import run_bass_kernel_spmd

B, NIN, NHID, NOUT = 4096, 2048, 4096, 1024
NC = 8
BS = B // NC  # 512 batch rows per core
KT = NIN // 128  # 16 k-tiles
NT = NOUT // 128  # 8 n-tiles
KSH = NIN // NC  # 256: beta_norm k-slice per core
HSH = NHID // NC  # 512: W2 h-slice per core
EPS = 0.1
F32 = mybir.dt.float32
F32R = mybir.dt.float32r

_CACHE = {}


def build_bass():
    nc = bacc.Bacc("TRN2", target_bir_lowering=False, debug=False, num_devices=NC)

    xT = nc.declare_dram_parameter("xT", [NIN, BS], F32, isOutput=False)
    yT = nc.declare_dram_parameter("yT", [NOUT, BS], F32, isOutput=False)
    bet = nc.declare_dram_parameter("beta", [NIN, NOUT], F32, isOutput=False)
    btp = nc.declare_dram_parameter("btp", [128, NT, KSH], F32, isOutput=False)
    w2p = nc.declare_dram_parameter("w2p", [128, NT, HSH], F32, isOutput=False)
    blp = nc.declare_dram_parameter("blp", [128, NT], F32, isOutput=False)
    b2p = nc.declare_dram_parameter("b2p", [128, NT], F32, isOutput=False)
    out = nc.declare_dram_parameter("out", [NOUT, BS], F32, isOutput=True)

    with (
        tile.TileContext(nc) as tc,
        tc.tile_pool(name="bsb", bufs=KT) as bpool,
        tc.tile_pool(name="xsb", bufs=KT) as xpool,
        tc.tile_pool(name="yts", bufs=NT) as ypool,
        tc.tile_pool(name="aux", bufs=1) as aux,
        tc.tile_pool(name="psum", bufs=1, space="PSUM") as ppool,
        tc.tile_pool(name="dram", bufs=1, space="DRAM") as dpool,
    ):
        ps = [
            ppool.tile([128, BS], F32, name=f"ps{n}", tag=f"ps{n}")
            for n in range(NT)
        ]

        # ---- Collective path FIRST, entirely on the scalar HWDGE ring so
        # it is never head-blocked by the streaming DMAs.  The AllReduce
        # trigger fires ~7us into each core's local time; cross-core start
        # skew then hides under the matmul stream.
        w2s = aux.tile([128, NT, HSH], F32)
        nc.scalar.dma_start(out=w2s[:], in_=w2p[:])
        bts = aux.tile([128, NT, KSH], F32)
        nc.scalar.dma_start(out=bts[:], in_=btp[:])
        part = aux.tile([128, 2 * NT], F32)
        nc.vector.tensor_reduce(
            out=part[:, 0:NT],
            in_=w2s[:],
            axis=mybir.AxisListType.X,
            op=mybir.AluOpType.add,
        )
        nc.vector.tensor_reduce(
            out=part[:, NT : 2 * NT],
            in_=bts[:],
            axis=mybir.AxisListType.X,
            op=mybir.AluOpType.add,
            apply_absolute_value=True,
        )
        cin = dpool.tile([128, 2 * NT], F32)
        cout = dpool.tile([128, 2 * NT], F32)
        nc.scalar.dma_start(out=cin[:], in_=part[:])
        nc.gpsimd.collective_compute(
            "AllReduce",
            mybir.AluOpType.add,
            replica_groups=[list(range(NC))],
            ins=[cin.opt()],
            outs=[cout.opt()],
        )
        allred = aux.tile([128, 2 * NT], F32)
        nc.scalar.dma_start(out=allred[:], in_=cout[:])

        # scale[n] = -EPS * beta_norm[n];  biasc[n] = NHID*w2sum + bias2 + bias_lin
        blt = aux.tile([128, NT], F32)
        nc.scalar.dma_start(out=blt[:], in_=blp[:])
        b2t = aux.tile([128, NT], F32)
        nc.scalar.dma_start(out=b2t[:], in_=b2p[:])
        scale = aux.tile([128, NT], F32)
        nc.vector.tensor_scalar_mul(scale[:], allred[:, NT : 2 * NT], -EPS)
        biasc = aux.tile([128, NT], F32)
        nc.vector.tensor_scalar_mul(biasc[:], allred[:, 0:NT], float(NHID))
        nc.vector.tensor_add(biasc[:], biasc[:], b2t[:])
        nc.vector.tensor_add(biasc[:], biasc[:], blt[:])

        # t[n] = yT*scale + biasc on the scalar engine; independent of the
        # matmuls, runs as soon as the collective lands.
        yts = []
        for n in range(NT):
            yt = ypool.tile([128, BS], F32, tag="yt")
            nc.scalar.dma_start(out=yt[:], in_=yT[n * 128 : (n + 1) * 128, :])
            nc.scalar.activation(
                yt[:],
                yt[:],
                mybir.ActivationFunctionType.Identity,
                bias=biasc[:, n : n + 1],
                scale=scale[:, n : n + 1],
            )
            yts.append(yt)

        # ---- Main matmul stream: k-outer / n-inner, beta+xT on the sync
        # ring, uninterrupted so the PE never idles (HAM stays warm).
        for k in range(KT):
            bt = bpool.tile([128, NOUT], F32R, tag="bt")
            nc.sync.dma_start(
                out=bt[:], in_=bet[k * 128 : (k + 1) * 128, :].bitcast(F32R)
            )
            xt = xpool.tile([128, BS], F32R, tag="xt")
            nc.sync.dma_start(
                out=xt[:], in_=xT[k * 128 : (k + 1) * 128, :].bitcast(F32R)
            )
            for n in range(NT):
                nc.tensor.matmul(
                    ps[n][:],
                    lhsT=bt[:, n * 128 : (n + 1) * 128],
                    rhs=xt[:],
                    start=(k == 0),
                    stop=(k == KT - 1),
                )

        # Epilogue: out = psum(lin^T) + t, then store.
        for n in range(NT):
            nc.vector.tensor_add(yts[n][:], ps[n][:], yts[n][:])
            nc.scalar.dma_start(out=out[n * 128 : (n + 1) * 128, :], in_=yts[n][:])

    nc.compile()
    return nc


def _get_nc():
    if "nc" not in _CACHE:
        _CACHE["nc"] = build_bass()
    return _CACHE["nc"]


def _shard_inputs(x, y, beta, bias_lin, W2, bias2):
    x = np.ascontiguousarray(x, dtype=np.float32)
    y = np.ascontiguousarray(y, dtype=np.float32)
    beta = np.ascontiguousarray(beta, dtype=np.float32)
    W2 = np.ascontiguousarray(W2, dtype=np.float32)
    blp = np.ascontiguousarray(np.asarray(bias_lin, np.float32).reshape(NT, 128).T)
    b2p = np.ascontiguousarray(np.asarray(bias2, np.float32).reshape(NT, 128).T)
    betaT = np.ascontiguousarray(beta.T)  # [NOUT, NIN]
    in_maps = []
    for c in range(NC):
        bsl = slice(c * BS, (c + 1) * BS)
        # [128, NT, KSH]: btp[p, t, k] = |slice later| beta[c*KSH+k, t*128+p]
        btp = np.ascontiguousarray(
            betaT[:, c * KSH : (c + 1) * KSH]
            .reshape(NT, 128, KSH)
            .transpose(1, 0, 2)
        )
        w2p = np.ascontiguousarray(
            W2[:, c * HSH : (c + 1) * HSH].reshape(NT, 128, HSH).transpose(1, 0, 2)
        )
        in_maps.append(
            {
                "xT": np.ascontiguousarray(x[bsl].T),
                "yT": np.ascontiguousarray(y[bsl].T),
                "beta": beta,
                "btp": btp,
                "w2p": w2p,
                "blp": blp,
                "b2p": b2p,
            }
        )
    return in_maps


def run_device(inputs, trace=False, **kw):
    nc = _get_nc()
    in_maps = _shard_inputs(
        inputs["x"], inputs["y"], inputs["beta"], inputs["bias_lin"],
        inputs["W2"], inputs["bias2"],
    )
    res = run_bass_kernel_spmd(nc, in_maps, core_ids=list(range(NC)), trace=trace, **kw)
    full = np.empty((B, NOUT), dtype=np.float32)
    for c in range(NC):
        full[c * BS : (c + 1) * BS, :] = res.results[c]["out"].T
    return full, res


def _reference_numpy(x, y, beta, bias_lin, W1, W2, bias1, bias2, adv):
    # Fallback for the adv=0 path (never produced by setup_inputs).
    x = np.asarray(x, np.float32)
    lin = x @ np.asarray(beta, np.float32) + np.asarray(bias_lin, np.float32)
    if adv:
        beta_norm = np.sum(np.abs(np.asarray(beta, np.float32)), axis=0)
        lin = lin - EPS * np.asarray(y, np.float32) * beta_norm
        one = NHID * np.sum(np.asarray(W2, np.float32), axis=1) + np.asarray(
            bias2, np.float32
        )
        one = np.broadcast_to(one, lin.shape)
    else:
        h = np.maximum(
            x @ np.asarray(W1, np.float32).T + np.asarray(bias1, np.float32), 0.0
        )
        one = h @ np.asarray(W2, np.float32).T + np.asarray(bias2, np.float32)
    return (lin + one).astype(np.float32)


def kernel(**inputs) -> np.ndarray:
    adv = int(np.asarray(inputs.get("adv", 1)))
    if adv == 0:
        return _reference_numpy(
            inputs["x"], inputs["y"], inputs["beta"], inputs["bias_lin"],
            inputs["W1"], inputs["W2"], inputs["bias1"], inputs["bias2"], adv,
        )
    full, _ = run_device(inputs)
    return full
